# revision 11
# baseline (speedup 1.0000x reference)
"""CPC predictor loss kernel for 8x Trainium2 NeuronCores (Bass/Tile).

Strategy
--------
B=144 = 12 speakers x 12 utts; K=12 prediction steps; L=128; D=C=256.
Negatives are sampled within-speaker, so all scores for one speaker come
from the within-speaker Gram-style matrix

    M_su[l, b*128+j] = Wc[s,u,l,:] . z4[s,b,j,:]   (Wc = c @ W_k^T + b_k)

Work is sharded over 8 cores as 4 speaker-groups (3 speakers) x 2
k-groups (6 steps) = 18 (s,k) units per core, zero collectives.

Per (s,k) unit on-device:
  stage 1 (PE):   WcT[d, (u,l)] = W_k^T-tiles @ cT   (fp32r matmuls)
  stage 2 (PE):   M_su = WcT_u^T @ z-window-T          (fp32r matmuls)
  extract (GPSIMD ap_gather): per-16-partition-group index lists pull the
      11 class scores per row into a "fat" [128, 11*16] tile
  compact (DVE):  mask-multiply by sel16 + reduce -> f[128, 11]
  CE (DVE+ACT):   log-softmax over the 11 classes, argmax==0 indicator,
      accumulated into per-partition partials [128, 24]
  final:          ones^T @ partials matmul -> [1,24] -> DRAM

Host: pre-transposes all inputs into SBUF-ready layouts, precomputes the
RNG-derived gather indices (jax threefry on CPU - identical to the
reference's), and sums the 8 per-core partial vectors.
"""

import numpy as np

S, U, K, L, D, C, T = 12, 12, 12, 128, 256, 256, 140
N_CORES = 8
S_GRP, K_GRP = 4, 2          # core = sg * K_GRP + kg
S_PER, K_PER = 3, 6          # speakers / k-steps per core
ACT_COLS = 1024              # M psum->sbuf copy: ACT [0:ACT_COLS), DVE rest

_CACHE: dict = {}


def _build_indices(num_negatives: int):
    """Reproduce the reference's negative-sampling indices (jax threefry)."""
    import jax
    import jax.numpy as jnp

    cpu = jax.devices("cpu")[0]
    with jax.default_device(cpu):
        base = jax.random.key(42)
        bidx_all = np.zeros((K, U, num_negatives), np.int64)
        sidx_all = np.zeros((K, S, U, num_negatives, L), np.int64)
        for k in range(K):
            kb, ks = jax.random.split(jax.random.fold_in(base, k))
            bidx = jax.random.randint(kb, (U, num_negatives), 0, U)
            sidx = jax.random.randint(ks, (S, U, num_negatives, L), 0, L)
            sidx = (sidx + jnp.arange(L)) % L
            bidx_all[k] = np.asarray(bidx)
            sidx_all[k] = np.asarray(sidx)

    # gather columns: col(s,k,u,l,0) = u*L + l (positive);
    # col(s,k,u,l,1+n) = bidx[k,u,n]*L + sidx[k,s,u,n,l]
    NN = num_negatives + 1
    idx = np.zeros((S, K, U, L, NN), np.int16)
    lr = np.arange(L)
    for s in range(S):
        for k in range(K):
            for u in range(U):
                idx[s, k, u, :, 0] = u * L + lr
                idx[s, k, u, :, 1:] = (
                    bidx_all[k, u][None, :] * L + sidx_all[k, s, u].T
                )
    return idx


def _build_program(num_negatives: int):
    import concourse.bass as bass
    import concourse.bacc as bacc
    import concourse.tile as tile
    from concourse import mybir
    from contextlib import ExitStack

    NN = num_negatives + 1
    FAT = NN * 16
    f32 = mybir.dt.float32
    f32r = mybir.dt.float32r
    i16 = mybir.dt.int16

    nc = bacc.Bacc("TRN2", target_bir_lowering=False, debug=False,
                   num_devices=N_CORES)

    zT_d = nc.dram_tensor("zT", (2, 128, S_PER, U, T), f32r,
                          kind="ExternalInput").ap()
    cT_d = nc.dram_tensor("cT", (2, 128, S_PER, U, L), f32r,
                          kind="ExternalInput").ap()
    WT_d = nc.dram_tensor("WT", (K_PER, 2, 128, 256), f32r,
                          kind="ExternalInput").ap()
    b_d = nc.dram_tensor("bT", (128, K_PER, 2), f32,
                         kind="ExternalInput").ap()
    gidx_d = nc.dram_tensor("gidx", (128, S_PER, K_PER, U, NN), i16,
                            kind="ExternalInput").ap()
    sel_d = nc.dram_tensor("sel16", (128, FAT), f32,
                           kind="ExternalInput").ap()
    out_d = nc.dram_tensor("out_part", (1, 24), f32,
                           kind="ExternalOutput").ap()

    with ExitStack() as ctx:
        tc = ctx.enter_context(tile.TileContext(nc))
        const = ctx.enter_context(tc.tile_pool(name="const", bufs=1))
        zpool = ctx.enter_context(tc.tile_pool(name="zpool", bufs=2))
        wc_pool = ctx.enter_context(tc.tile_pool(name="wc", bufs=2))
        m_pool = ctx.enter_context(tc.tile_pool(name="m", bufs=3))
        fat_pool = ctx.enter_context(tc.tile_pool(name="fat", bufs=3))
        small = ctx.enter_context(tc.tile_pool(name="small", bufs=2))
        acc_pool = ctx.enter_context(tc.tile_pool(name="acc", bufs=1))
        psum_m = ctx.enter_context(tc.tile_pool(name="psum_m", bufs=2,
                                                space="PSUM"))

        # --- constants / whole-run inputs ---
        from concourse import library_config
        nc.gpsimd.load_library(library_config.ap_gather)

        w_sb = const.tile([128, K_PER, 2, 256], f32r)      # [c_, k, kc, d]
        for kl in range(K_PER):
            for kc in range(2):
                nc.sync.dma_start(out=w_sb[:, kl, kc, :],
                                  in_=WT_d[kl, kc])
        b_sb = const.tile([128, K_PER, 2], f32)
        nc.sync.dma_start(out=b_sb, in_=b_d)
        gidx_sb = const.tile([128, S_PER, K_PER, U, NN], i16)
        nc.sync.dma_start(out=gidx_sb, in_=gidx_d)
        sel_sb = const.tile([128, FAT], f32)
        nc.sync.dma_start(out=sel_sb, in_=sel_d)
        sel3 = sel_sb.rearrange("p (n j) -> p n j", j=16)
        ones_sb = const.tile([128, 1], f32)
        nc.vector.memset(ones_sb, 1.0)

        partials = acc_pool.tile([128, 24], f32)
        nc.vector.memset(partials, 0.0)
        ssum_all = acc_pool.tile([128, S_PER * K_PER, U], f32)

        for s in range(S_PER):
            zt = {}
            ct = {}
            for dh in range(2):
                z_tile = zpool.tile([128, U, T], f32r, tag=f"zT{dh}",
                                    name=f"z_s{s}_d{dh}")
                nc.sync.dma_start(out=z_tile, in_=zT_d[dh, :, s])
                zt[dh] = z_tile
                c_tile = zpool.tile([128, U, L], f32r, tag=f"cT{dh}",
                                    name=f"c_s{s}_d{dh}")
                nc.sync.dma_start(out=c_tile, in_=cT_d[dh, :, s])
                ct[dh] = c_tile

            for kl in range(K_PER):
                # ---- stage 1: WcT[d-half, 1536 tokens] ----
                wc = {}
                for dh in range(2):
                    wt = wc_pool.tile([128, U * L], f32r, tag=f"wc{dh}",
                                      name=f"wc_s{s}_k{kl}_d{dh}")
                    wc[dh] = wt
                    pmw = psum_m.tile([128, U * L], f32, tag="m",
                                      name=f"pwc_{s}_{kl}_{dh}")
                    for kc in range(2):
                        for nt in range(3):
                            rhs = (ct[kc].rearrange("p u t -> p (u t)")
                                   [:, nt * 512:(nt + 1) * 512])
                            nc.tensor.matmul(
                                pmw[:, nt * 512:(nt + 1) * 512],
                                lhsT=w_sb[:, kl, kc,
                                          dh * 128:(dh + 1) * 128],
                                rhs=rhs,
                                start=(kc == 0), stop=(kc == 1),
                            )
                    # bias-add + psum->sbuf (ACT, Identity w/ bias)
                    nc.scalar.activation(
                        wt, pmw,
                        mybir.ActivationFunctionType.Identity,
                        bias=b_sb[:, kl, dh:dh + 1],
                    )

                f_all = small.tile([128, U, NN], f32, tag="f_all",
                                   name=f"f_{s}_{kl}")
                # ---- stage 2 + extract per u ----
                for u in range(U):
                    pm = psum_m.tile([128, U * L], f32, tag="m",
                                     name=f"pm_{s}_{kl}_{u}")
                    for dh in range(2):
                        for bt in range(3):
                            rhs = zt[dh][:, bt * 4:(bt + 1) * 4,
                                         kl + 1:kl + 1 + L]
                            nc.tensor.matmul(
                                pm[:, bt * 512:(bt + 1) * 512],
                                lhsT=wc[dh][:, u * L:(u + 1) * L],
                                rhs=rhs,
                                start=(dh == 0), stop=(dh == 1),
                            )
                    m_sb = m_pool.tile([128, U * L], f32, tag="m_sb",
                                       name=f"m_{s}_{kl}_{u}")
                    nc.scalar.copy(m_sb[:, :ACT_COLS],
                                   pm[:, :ACT_COLS])
                    nc.vector.tensor_copy(m_sb[:, ACT_COLS:],
                                          pm[:, ACT_COLS:])
                    f_fat = fat_pool.tile([128, FAT], f32, tag="fat",
                                          name=f"ff_{s}_{kl}_{u}")
                    nc.gpsimd.ap_gather(
                        f_fat, m_sb, gidx_sb[:, s, kl, u, :],
                        channels=128, num_elems=U * L, d=1, num_idxs=FAT,
                    )
                    # compact: f_all[:, u, :] = sum_j fat*sel16
                    fm = fat_pool.tile([128, NN, 16], f32, tag="fm",
                                       name=f"fm_{s}_{kl}_{u}")
                    nc.vector.tensor_mul(
                        fm, f_fat.rearrange("p (n j) -> p n j", j=16), sel3)
                    nc.vector.reduce_sum(f_all[:, u, :], fm,
                                         axis=mybir.AxisListType.X)

                # ---- softmax-CE over the NN classes, batched over u ----
                kcol = kl  # local k; host maps to global k
                fmax = small.tile([128, U], f32, tag="fmax",
                                  name=f"fmax_{s}_{kl}")
                nc.vector.reduce_max(fmax, f_all,
                                     axis=mybir.AxisListType.X)
                ts_t = small.tile([128, U, NN], f32, tag="ts",
                                  name=f"ts_{s}_{kl}")
                fmax_b = bass.AP(tensor=fmax.tensor, offset=fmax.offset,
                                 ap=[*fmax.ap, [0, NN]])
                nc.vector.tensor_sub(ts_t, f_all, fmax_b)
                e_t = small.tile([128, U, NN], f32, tag="e",
                                 name=f"e_{s}_{kl}")
                nc.scalar.activation(e_t, ts_t,
                                     mybir.ActivationFunctionType.Exp)
                unit = s * K_PER + kl
                nc.vector.reduce_sum(ssum_all[:, unit, :], e_t,
                                     axis=mybir.AxisListType.X)
                lp_s = small.tile([128, 1], f32, tag="lp_s",
                                  name=f"lps_{s}_{kl}")
                nc.vector.reduce_sum(lp_s, ts_t[:, :, 0],
                                     axis=mybir.AxisListType.X)
                nc.vector.tensor_add(partials[:, kcol:kcol + 1],
                                     partials[:, kcol:kcol + 1], lp_s)
                # acc: f0 >= max(negatives)
                nmax = small.tile([128, U], f32, tag="nmax",
                                  name=f"nm_{s}_{kl}")
                nc.vector.reduce_max(nmax, f_all[:, :, 1:],
                                     axis=mybir.AxisListType.X)
                ind = small.tile([128, U], f32, tag="ind",
                                 name=f"ind_{s}_{kl}")
                nc.vector.tensor_tensor(ind, f_all[:, :, 0], nmax,
                                        op=mybir.AluOpType.is_ge)
                ind_s = small.tile([128, 1], f32, tag="ind_s",
                                   name=f"inds_{s}_{kl}")
                nc.vector.reduce_sum(ind_s, ind,
                                     axis=mybir.AxisListType.X)
                nc.vector.tensor_add(partials[:, 12 + kcol:13 + kcol],
                                     partials[:, 12 + kcol:13 + kcol],
                                     ind_s)

        # ---- deferred Ln over all units' softmax sums ----
        lse_all = acc_pool.tile([128, S_PER * K_PER, U], f32)
        nc.scalar.activation(
            lse_all.rearrange("p a u -> p (a u)"),
            ssum_all.rearrange("p a u -> p (a u)"),
            mybir.ActivationFunctionType.Ln)
        for s in range(S_PER):
            for kl in range(K_PER):
                unit = s * K_PER + kl
                ls_s = small.tile([128, 1], f32, tag="ls_s",
                                  name=f"lss_{s}_{kl}")
                nc.vector.reduce_sum(ls_s, lse_all[:, unit, :],
                                     axis=mybir.AxisListType.X)
                nc.vector.tensor_sub(partials[:, kl:kl + 1],
                                     partials[:, kl:kl + 1], ls_s)

        # ---- final partition reduce: ones^T @ partials -> [1, 24] ----
        po = psum_m.tile([1, 24], f32, tag="m")
        nc.tensor.matmul(po, lhsT=ones_sb, rhs=partials,
                         start=True, stop=True)
        out_sb = small.tile([1, 24], f32, tag="out_sb")
        nc.vector.tensor_copy(out_sb, po)
        nc.sync.dma_start(out=out_d, in_=out_sb)

    nc.compile()
    return nc


def _prep_inputs(z, c, W, b, idx, num_negatives):
    """Build the 8 per-core input dicts (all numpy, host-side only)."""
    NN = num_negatives + 1
    FAT = NN * 16
    scale = np.float32(1.0 / np.sqrt(np.float32(D)))
    Ws = (W * scale).astype(np.float32)
    bs = (b * scale).astype(np.float32)

    sel16 = np.zeros((128, FAT), np.float32)
    pr = np.arange(128)
    for n in range(NN):
        sel16[pr, n * 16 + (pr % 16)] = 1.0

    in_maps = []
    for core in range(N_CORES):
        sg, kg = core // K_GRP, core % K_GRP
        sp = slice(sg * S_PER * U, (sg + 1) * S_PER * U)
        ks = slice(kg * K_PER, (kg + 1) * K_PER)
        # Pre-shift the time axis by the core's k-group offset so the
        # device's local-k window [kl+1, kl+1+L) lands on the global
        # window [kg*K_PER + kl + 1, ...).
        zsh = np.zeros((S_PER * U, T, D), z.dtype)
        zsh[:, :T - kg * K_PER] = z[sp, kg * K_PER:]
        zT = (zsh.reshape(S_PER, U, T, D).transpose(3, 0, 1, 2)
              .reshape(2, 128, S_PER, U, T))
        cT = (c[sp, :L].reshape(S_PER, U, L, C).transpose(3, 0, 1, 2)
              .reshape(2, 128, S_PER, U, L))
        WT = Ws[ks].transpose(0, 2, 1).reshape(K_PER, 2, 128, 256)
        bT = bs[ks].reshape(K_PER, 2, 128).transpose(2, 0, 1)
        gidx = idx[sg * S_PER:(sg + 1) * S_PER, ks].transpose(3, 0, 1, 2, 4)
        in_maps.append({
            "zT": np.ascontiguousarray(zT),
            "cT": np.ascontiguousarray(cT),
            "WT": np.ascontiguousarray(WT),
            "bT": np.ascontiguousarray(bT),
            "gidx": np.ascontiguousarray(gidx),
            "sel16": sel16,
        })
    return in_maps


def _finalize(core_outs):
    # out_part columns use LOCAL k (0..5); map to global k via the core's
    # k-group: global_k = kg*K_PER + kl, kg = core % K_GRP.
    loss_k = np.zeros(K, np.float64)
    acc_k = np.zeros(K, np.float64)
    for core, o in enumerate(core_outs):
        kg = core % K_GRP
        v = np.asarray(o["out_part"], np.float64).reshape(24)
        loss_k[kg * K_PER:(kg + 1) * K_PER] += v[:K_PER]
        acc_k[kg * K_PER:(kg + 1) * K_PER] += v[12:12 + K_PER]
    loss = -loss_k.sum() / (K * S * U * L)
    accs = acc_k / (S * U * L)
    return np.float32(loss), accs.astype(np.float32)


def kernel(z, c, W, b, num_negatives):
    num_negatives = int(num_negatives)
    z = np.asarray(z, np.float32)
    c = np.asarray(c, np.float32)
    W = np.asarray(W, np.float32)
    b = np.asarray(b, np.float32)

    key = num_negatives
    if key not in _CACHE:
        idx = _build_indices(num_negatives)
        nc = _build_program(num_negatives)
        _CACHE[key] = (idx, nc)
    idx, nc = _CACHE[key]

    in_maps = _prep_inputs(z, c, W, b, idx, num_negatives)

    from concourse.bass_utils import run_bass_kernel_spmd
    res = run_bass_kernel_spmd(nc, in_maps, core_ids=list(range(N_CORES)))
    return _finalize(res.results)


if __name__ == "__main__":
    import reference
    inputs = reference.setup_inputs()
    el, ea = reference.reference(**inputs)
    al, aa = kernel(**inputs)
    print("loss:", al, "exp:", float(el),
          "relerr:", abs(al - float(el)) / abs(float(el)))
    print("accs maxrelerr:",
          np.abs(aa - np.asarray(ea)).max() / np.abs(np.asarray(ea)).max())


# revision 12
# speedup vs baseline: 1.0018x; 1.0018x over previous
"""CPC predictor loss kernel for 8x Trainium2 NeuronCores (Bass/Tile).

Strategy
--------
B=144 = 12 speakers x 12 utts; K=12 prediction steps; L=128; D=C=256.
Negatives are sampled within-speaker, so all scores for one speaker come
from the within-speaker Gram-style matrix

    M_su[l, b*128+j] = Wc[s,u,l,:] . z4[s,b,j,:]   (Wc = c @ W_k^T + b_k)

Work is sharded over 8 cores as 4 speaker-groups (3 speakers) x 2
k-groups (6 steps) = 18 (s,k) units per core, zero collectives.

Per (s,k) unit on-device:
  stage 1 (PE):   WcT[d, (u,l)] = W_k^T-tiles @ cT   (fp32r matmuls)
  stage 2 (PE):   M_su = WcT_u^T @ z-window-T          (fp32r matmuls)
  extract (GPSIMD ap_gather): per-16-partition-group index lists pull the
      11 class scores per row into a "fat" [128, 11*16] tile
  compact (DVE):  mask-multiply by sel16 + reduce -> f[128, 11]
  CE (DVE+ACT):   log-softmax over the 11 classes, argmax==0 indicator,
      accumulated into per-partition partials [128, 24]
  final:          ones^T @ partials matmul -> [1,24] -> DRAM

Host: pre-transposes all inputs into SBUF-ready layouts, precomputes the
RNG-derived gather indices (jax threefry on CPU - identical to the
reference's), and sums the 8 per-core partial vectors.
"""

import numpy as np

S, U, K, L, D, C, T = 12, 12, 12, 128, 256, 256, 140
N_CORES = 8
S_GRP, K_GRP = 4, 2          # core = sg * K_GRP + kg
S_PER, K_PER = 3, 6          # speakers / k-steps per core
ACT_COLS = 1024              # M psum->sbuf copy: ACT [0:ACT_COLS), DVE rest

_CACHE: dict = {}


def _build_indices(num_negatives: int):
    """Reproduce the reference's negative-sampling indices (jax threefry)."""
    import jax
    import jax.numpy as jnp

    cpu = jax.devices("cpu")[0]
    with jax.default_device(cpu):
        base = jax.random.key(42)
        bidx_all = np.zeros((K, U, num_negatives), np.int64)
        sidx_all = np.zeros((K, S, U, num_negatives, L), np.int64)
        for k in range(K):
            kb, ks = jax.random.split(jax.random.fold_in(base, k))
            bidx = jax.random.randint(kb, (U, num_negatives), 0, U)
            sidx = jax.random.randint(ks, (S, U, num_negatives, L), 0, L)
            sidx = (sidx + jnp.arange(L)) % L
            bidx_all[k] = np.asarray(bidx)
            sidx_all[k] = np.asarray(sidx)

    # gather columns: col(s,k,u,l,0) = u*L + l (positive);
    # col(s,k,u,l,1+n) = bidx[k,u,n]*L + sidx[k,s,u,n,l]
    NN = num_negatives + 1
    idx = np.zeros((S, K, U, L, NN), np.int16)
    lr = np.arange(L)
    for s in range(S):
        for k in range(K):
            for u in range(U):
                idx[s, k, u, :, 0] = u * L + lr
                idx[s, k, u, :, 1:] = (
                    bidx_all[k, u][None, :] * L + sidx_all[k, s, u].T
                )
    return idx


def _build_program(num_negatives: int):
    import concourse.bass as bass
    import concourse.bacc as bacc
    import concourse.tile as tile
    from concourse import mybir
    from contextlib import ExitStack

    NN = num_negatives + 1
    FAT = NN * 16
    f32 = mybir.dt.float32
    f32r = mybir.dt.float32r
    bf = mybir.dt.bfloat16
    i16 = mybir.dt.int16

    nc = bacc.Bacc("TRN2", target_bir_lowering=False, debug=False,
                   num_devices=N_CORES)

    zT_d = nc.dram_tensor("zT", (2, 128, S_PER, U, T), bf,
                          kind="ExternalInput").ap()
    cT_d = nc.dram_tensor("cT", (2, 128, S_PER, U, L), bf,
                          kind="ExternalInput").ap()
    WT_d = nc.dram_tensor("WT", (K_PER, 2, 128, 256), bf,
                          kind="ExternalInput").ap()
    b_d = nc.dram_tensor("bT", (128, K_PER, 2), f32,
                         kind="ExternalInput").ap()
    gidx_d = nc.dram_tensor("gidx", (128, S_PER, K_PER, U, NN), i16,
                            kind="ExternalInput").ap()
    sel_d = nc.dram_tensor("sel16", (128, FAT), f32,
                           kind="ExternalInput").ap()
    out_d = nc.dram_tensor("out_part", (1, 24), f32,
                           kind="ExternalOutput").ap()

    with ExitStack() as ctx:
        tc = ctx.enter_context(tile.TileContext(nc))
        const = ctx.enter_context(tc.tile_pool(name="const", bufs=1))
        zpool = ctx.enter_context(tc.tile_pool(name="zpool", bufs=2))
        wc_pool = ctx.enter_context(tc.tile_pool(name="wc", bufs=2))
        m_pool = ctx.enter_context(tc.tile_pool(name="m", bufs=3))
        fat_pool = ctx.enter_context(tc.tile_pool(name="fat", bufs=3))
        small = ctx.enter_context(tc.tile_pool(name="small", bufs=2))
        acc_pool = ctx.enter_context(tc.tile_pool(name="acc", bufs=1))
        psum_m = ctx.enter_context(tc.tile_pool(name="psum_m", bufs=2,
                                                space="PSUM"))

        # --- constants / whole-run inputs ---
        from concourse import library_config
        nc.gpsimd.load_library(library_config.ap_gather)

        w_sb = const.tile([128, K_PER, 2, 256], bf)      # [c_, k, kc, d]
        for kl in range(K_PER):
            for kc in range(2):
                nc.sync.dma_start(out=w_sb[:, kl, kc, :],
                                  in_=WT_d[kl, kc])
        b_sb = const.tile([128, K_PER, 2], f32)
        nc.sync.dma_start(out=b_sb, in_=b_d)
        gidx_sb = const.tile([128, S_PER, K_PER, U, NN], i16)
        nc.sync.dma_start(out=gidx_sb, in_=gidx_d)
        sel_sb = const.tile([128, FAT], f32)
        nc.sync.dma_start(out=sel_sb, in_=sel_d)
        sel3 = sel_sb.rearrange("p (n j) -> p n j", j=16)
        ones_sb = const.tile([128, 1], f32)
        nc.vector.memset(ones_sb, 1.0)

        partials = acc_pool.tile([128, 24], f32)
        nc.vector.memset(partials, 0.0)
        ssum_all = acc_pool.tile([128, S_PER * K_PER, U], f32)

        for s in range(S_PER):
            zt = {}
            ct = {}
            for dh in range(2):
                z_tile = zpool.tile([128, U, T], bf, tag=f"zT{dh}",
                                    name=f"z_s{s}_d{dh}")
                nc.sync.dma_start(out=z_tile, in_=zT_d[dh, :, s])
                zt[dh] = z_tile
                c_tile = zpool.tile([128, U, L], bf, tag=f"cT{dh}",
                                    name=f"c_s{s}_d{dh}")
                nc.sync.dma_start(out=c_tile, in_=cT_d[dh, :, s])
                ct[dh] = c_tile

            for kl in range(K_PER):
                # ---- stage 1: WcT[d-half, 1536 tokens] ----
                wc = {}
                for dh in range(2):
                    wt = wc_pool.tile([128, U * L], bf, tag=f"wc{dh}",
                                      name=f"wc_s{s}_k{kl}_d{dh}")
                    wc[dh] = wt
                    pmw = psum_m.tile([128, U * L], f32, tag="m",
                                      name=f"pwc_{s}_{kl}_{dh}")
                    for kc in range(2):
                        for nt in range(3):
                            rhs = (ct[kc].rearrange("p u t -> p (u t)")
                                   [:, nt * 512:(nt + 1) * 512])
                            nc.tensor.matmul(
                                pmw[:, nt * 512:(nt + 1) * 512],
                                lhsT=w_sb[:, kl, kc,
                                          dh * 128:(dh + 1) * 128],
                                rhs=rhs,
                                start=(kc == 0), stop=(kc == 1),
                            )
                    # bias-add + psum->sbuf (ACT, Identity w/ bias)
                    nc.scalar.activation(
                        wt, pmw,
                        mybir.ActivationFunctionType.Identity,
                        bias=b_sb[:, kl, dh:dh + 1],
                    )

                f_all = small.tile([128, U, NN], f32, tag="f_all",
                                   name=f"f_{s}_{kl}")
                # ---- stage 2 + extract per u ----
                for u in range(U):
                    pm = psum_m.tile([128, U * L], f32, tag="m",
                                     name=f"pm_{s}_{kl}_{u}")
                    for dh in range(2):
                        for bt in range(3):
                            rhs = zt[dh][:, bt * 4:(bt + 1) * 4,
                                         kl + 1:kl + 1 + L]
                            nc.tensor.matmul(
                                pm[:, bt * 512:(bt + 1) * 512],
                                lhsT=wc[dh][:, u * L:(u + 1) * L],
                                rhs=rhs,
                                start=(dh == 0), stop=(dh == 1),
                            )
                    m_sb = m_pool.tile([128, U * L], f32, tag="m_sb",
                                       name=f"m_{s}_{kl}_{u}")
                    nc.scalar.copy(m_sb[:, :ACT_COLS],
                                   pm[:, :ACT_COLS])
                    nc.vector.tensor_copy(m_sb[:, ACT_COLS:],
                                          pm[:, ACT_COLS:])
                    f_fat = fat_pool.tile([128, FAT], f32, tag="fat",
                                          name=f"ff_{s}_{kl}_{u}")
                    nc.gpsimd.ap_gather(
                        f_fat, m_sb, gidx_sb[:, s, kl, u, :],
                        channels=128, num_elems=U * L, d=1, num_idxs=FAT,
                    )
                    # compact: f_all[:, u, :] = sum_j fat*sel16
                    fm = fat_pool.tile([128, NN, 16], f32, tag="fm",
                                       name=f"fm_{s}_{kl}_{u}")
                    nc.vector.tensor_mul(
                        fm, f_fat.rearrange("p (n j) -> p n j", j=16), sel3)
                    nc.vector.reduce_sum(f_all[:, u, :], fm,
                                         axis=mybir.AxisListType.X)

                # ---- softmax-CE over the NN classes, batched over u ----
                kcol = kl  # local k; host maps to global k
                fmax = small.tile([128, U], f32, tag="fmax",
                                  name=f"fmax_{s}_{kl}")
                nc.vector.reduce_max(fmax, f_all,
                                     axis=mybir.AxisListType.X)
                ts_t = small.tile([128, U, NN], f32, tag="ts",
                                  name=f"ts_{s}_{kl}")
                fmax_b = bass.AP(tensor=fmax.tensor, offset=fmax.offset,
                                 ap=[*fmax.ap, [0, NN]])
                nc.vector.tensor_sub(ts_t, f_all, fmax_b)
                e_t = small.tile([128, U, NN], f32, tag="e",
                                 name=f"e_{s}_{kl}")
                nc.scalar.activation(e_t, ts_t,
                                     mybir.ActivationFunctionType.Exp)
                unit = s * K_PER + kl
                nc.vector.reduce_sum(ssum_all[:, unit, :], e_t,
                                     axis=mybir.AxisListType.X)
                lp_s = small.tile([128, 1], f32, tag="lp_s",
                                  name=f"lps_{s}_{kl}")
                nc.vector.reduce_sum(lp_s, ts_t[:, :, 0],
                                     axis=mybir.AxisListType.X)
                nc.vector.tensor_add(partials[:, kcol:kcol + 1],
                                     partials[:, kcol:kcol + 1], lp_s)
                # acc: f0 >= max(negatives)
                nmax = small.tile([128, U], f32, tag="nmax",
                                  name=f"nm_{s}_{kl}")
                nc.vector.reduce_max(nmax, f_all[:, :, 1:],
                                     axis=mybir.AxisListType.X)
                ind = small.tile([128, U], f32, tag="ind",
                                 name=f"ind_{s}_{kl}")
                nc.vector.tensor_tensor(ind, f_all[:, :, 0], nmax,
                                        op=mybir.AluOpType.is_ge)
                ind_s = small.tile([128, 1], f32, tag="ind_s",
                                   name=f"inds_{s}_{kl}")
                nc.vector.reduce_sum(ind_s, ind,
                                     axis=mybir.AxisListType.X)
                nc.vector.tensor_add(partials[:, 12 + kcol:13 + kcol],
                                     partials[:, 12 + kcol:13 + kcol],
                                     ind_s)

        # ---- deferred Ln over all units' softmax sums ----
        lse_all = acc_pool.tile([128, S_PER * K_PER, U], f32)
        nc.scalar.activation(
            lse_all.rearrange("p a u -> p (a u)"),
            ssum_all.rearrange("p a u -> p (a u)"),
            mybir.ActivationFunctionType.Ln)
        for s in range(S_PER):
            for kl in range(K_PER):
                unit = s * K_PER + kl
                ls_s = small.tile([128, 1], f32, tag="ls_s",
                                  name=f"lss_{s}_{kl}")
                nc.vector.reduce_sum(ls_s, lse_all[:, unit, :],
                                     axis=mybir.AxisListType.X)
                nc.vector.tensor_sub(partials[:, kl:kl + 1],
                                     partials[:, kl:kl + 1], ls_s)

        # ---- final partition reduce: ones^T @ partials -> [1, 24] ----
        po = psum_m.tile([1, 24], f32, tag="m")
        nc.tensor.matmul(po, lhsT=ones_sb, rhs=partials,
                         start=True, stop=True)
        out_sb = small.tile([1, 24], f32, tag="out_sb")
        nc.vector.tensor_copy(out_sb, po)
        nc.sync.dma_start(out=out_d, in_=out_sb)

    nc.compile()
    return nc


def _prep_inputs(z, c, W, b, idx, num_negatives):
    """Build the 8 per-core input dicts (all numpy, host-side only)."""
    NN = num_negatives + 1
    FAT = NN * 16
    scale = np.float32(1.0 / np.sqrt(np.float32(D)))
    Ws = (W * scale).astype(np.float32)
    bs = (b * scale).astype(np.float32)

    sel16 = np.zeros((128, FAT), np.float32)
    pr = np.arange(128)
    for n in range(NN):
        sel16[pr, n * 16 + (pr % 16)] = 1.0

    in_maps = []
    for core in range(N_CORES):
        sg, kg = core // K_GRP, core % K_GRP
        sp = slice(sg * S_PER * U, (sg + 1) * S_PER * U)
        ks = slice(kg * K_PER, (kg + 1) * K_PER)
        # Pre-shift the time axis by the core's k-group offset so the
        # device's local-k window [kl+1, kl+1+L) lands on the global
        # window [kg*K_PER + kl + 1, ...).
        zsh = np.zeros((S_PER * U, T, D), z.dtype)
        zsh[:, :T - kg * K_PER] = z[sp, kg * K_PER:]
        zT = (zsh.reshape(S_PER, U, T, D).transpose(3, 0, 1, 2)
              .reshape(2, 128, S_PER, U, T))
        cT = (c[sp, :L].reshape(S_PER, U, L, C).transpose(3, 0, 1, 2)
              .reshape(2, 128, S_PER, U, L))
        WT = Ws[ks].transpose(0, 2, 1).reshape(K_PER, 2, 128, 256)
        bT = bs[ks].reshape(K_PER, 2, 128).transpose(2, 0, 1)
        gidx = idx[sg * S_PER:(sg + 1) * S_PER, ks].transpose(3, 0, 1, 2, 4)
        import ml_dtypes
        in_maps.append({
            "zT": np.ascontiguousarray(zT).astype(ml_dtypes.bfloat16),
            "cT": np.ascontiguousarray(cT).astype(ml_dtypes.bfloat16),
            "WT": np.ascontiguousarray(WT).astype(ml_dtypes.bfloat16),
            "bT": np.ascontiguousarray(bT),
            "gidx": np.ascontiguousarray(gidx),
            "sel16": sel16,
        })
    return in_maps


def _finalize(core_outs):
    # out_part columns use LOCAL k (0..5); map to global k via the core's
    # k-group: global_k = kg*K_PER + kl, kg = core % K_GRP.
    loss_k = np.zeros(K, np.float64)
    acc_k = np.zeros(K, np.float64)
    for core, o in enumerate(core_outs):
        kg = core % K_GRP
        v = np.asarray(o["out_part"], np.float64).reshape(24)
        loss_k[kg * K_PER:(kg + 1) * K_PER] += v[:K_PER]
        acc_k[kg * K_PER:(kg + 1) * K_PER] += v[12:12 + K_PER]
    loss = -loss_k.sum() / (K * S * U * L)
    accs = acc_k / (S * U * L)
    return np.float32(loss), accs.astype(np.float32)


def kernel(z, c, W, b, num_negatives):
    num_negatives = int(num_negatives)
    z = np.asarray(z, np.float32)
    c = np.asarray(c, np.float32)
    W = np.asarray(W, np.float32)
    b = np.asarray(b, np.float32)

    key = num_negatives
    if key not in _CACHE:
        idx = _build_indices(num_negatives)
        nc = _build_program(num_negatives)
        _CACHE[key] = (idx, nc)
    idx, nc = _CACHE[key]

    in_maps = _prep_inputs(z, c, W, b, idx, num_negatives)

    from concourse.bass_utils import run_bass_kernel_spmd
    res = run_bass_kernel_spmd(nc, in_maps, core_ids=list(range(N_CORES)))
    return _finalize(res.results)


if __name__ == "__main__":
    import reference
    inputs = reference.setup_inputs()
    el, ea = reference.reference(**inputs)
    al, aa = kernel(**inputs)
    print("loss:", al, "exp:", float(el),
          "relerr:", abs(al - float(el)) / abs(float(el)))
    print("accs maxrelerr:",
          np.abs(aa - np.asarray(ea)).max() / np.abs(np.asarray(ea)).max())


# revision 13
# speedup vs baseline: 1.0022x; 1.0003x over previous
"""CPC predictor loss kernel for 8x Trainium2 NeuronCores (Bass/Tile).

Strategy
--------
B=144 = 12 speakers x 12 utts; K=12 prediction steps; L=128; D=C=256.
Negatives are sampled within-speaker, so all scores for one speaker come
from the within-speaker Gram-style matrix

    M_su[l, b*128+j] = Wc[s,u,l,:] . z4[s,b,j,:]   (Wc = c @ W_k^T + b_k)

Work is sharded over 8 cores as 4 speaker-groups (3 speakers) x 2
k-groups (6 steps) = 18 (s,k) units per core, zero collectives.

Per (s,k) unit on-device:
  stage 1 (PE):   WcT[d, (u,l)] = W_k^T-tiles @ cT   (fp32r matmuls)
  stage 2 (PE):   M_su = WcT_u^T @ z-window-T          (fp32r matmuls)
  extract (GPSIMD ap_gather): per-16-partition-group index lists pull the
      11 class scores per row into a "fat" [128, 11*16] tile
  compact (DVE):  mask-multiply by sel16 + reduce -> f[128, 11]
  CE (DVE+ACT):   log-softmax over the 11 classes, argmax==0 indicator,
      accumulated into per-partition partials [128, 24]
  final:          ones^T @ partials matmul -> [1,24] -> DRAM

Host: pre-transposes all inputs into SBUF-ready layouts, precomputes the
RNG-derived gather indices (jax threefry on CPU - identical to the
reference's), and sums the 8 per-core partial vectors.
"""

import numpy as np

S, U, K, L, D, C, T = 12, 12, 12, 128, 256, 256, 140
N_CORES = 8
S_GRP, K_GRP = 4, 2          # core = sg * K_GRP + kg
S_PER, K_PER = 3, 6          # speakers / k-steps per core
ACT_COLS = 896               # M psum->sbuf copy: ACT [0:ACT_COLS), DVE rest

_CACHE: dict = {}


def _build_indices(num_negatives: int):
    """Reproduce the reference's negative-sampling indices (jax threefry)."""
    import jax
    import jax.numpy as jnp

    cpu = jax.devices("cpu")[0]
    with jax.default_device(cpu):
        base = jax.random.key(42)
        bidx_all = np.zeros((K, U, num_negatives), np.int64)
        sidx_all = np.zeros((K, S, U, num_negatives, L), np.int64)
        for k in range(K):
            kb, ks = jax.random.split(jax.random.fold_in(base, k))
            bidx = jax.random.randint(kb, (U, num_negatives), 0, U)
            sidx = jax.random.randint(ks, (S, U, num_negatives, L), 0, L)
            sidx = (sidx + jnp.arange(L)) % L
            bidx_all[k] = np.asarray(bidx)
            sidx_all[k] = np.asarray(sidx)

    # gather columns: col(s,k,u,l,0) = u*L + l (positive);
    # col(s,k,u,l,1+n) = bidx[k,u,n]*L + sidx[k,s,u,n,l]
    NN = num_negatives + 1
    idx = np.zeros((S, K, U, L, NN), np.int16)
    lr = np.arange(L)
    for s in range(S):
        for k in range(K):
            for u in range(U):
                idx[s, k, u, :, 0] = u * L + lr
                idx[s, k, u, :, 1:] = (
                    bidx_all[k, u][None, :] * L + sidx_all[k, s, u].T
                )
    return idx


def _build_program(num_negatives: int):
    import concourse.bass as bass
    import concourse.bacc as bacc
    import concourse.tile as tile
    from concourse import mybir
    from contextlib import ExitStack

    NN = num_negatives + 1
    FAT = NN * 16
    f32 = mybir.dt.float32
    f32r = mybir.dt.float32r
    bf = mybir.dt.bfloat16
    i16 = mybir.dt.int16

    nc = bacc.Bacc("TRN2", target_bir_lowering=False, debug=False,
                   num_devices=N_CORES)

    zT_d = nc.dram_tensor("zT", (2, 128, S_PER, U, T), bf,
                          kind="ExternalInput").ap()
    cT_d = nc.dram_tensor("cT", (2, 128, S_PER, U, L), bf,
                          kind="ExternalInput").ap()
    WT_d = nc.dram_tensor("WT", (K_PER, 2, 128, 256), bf,
                          kind="ExternalInput").ap()
    b_d = nc.dram_tensor("bT", (128, K_PER, 2), f32,
                         kind="ExternalInput").ap()
    gidx_d = nc.dram_tensor("gidx", (128, S_PER, K_PER, U, NN), i16,
                            kind="ExternalInput").ap()
    sel_d = nc.dram_tensor("sel16", (128, FAT), f32,
                           kind="ExternalInput").ap()
    out_d = nc.dram_tensor("out_part", (1, 24), f32,
                           kind="ExternalOutput").ap()

    with ExitStack() as ctx:
        tc = ctx.enter_context(tile.TileContext(nc))
        const = ctx.enter_context(tc.tile_pool(name="const", bufs=1))
        zpool = ctx.enter_context(tc.tile_pool(name="zpool", bufs=2))
        wc_pool = ctx.enter_context(tc.tile_pool(name="wc", bufs=2))
        m_pool = ctx.enter_context(tc.tile_pool(name="m", bufs=4))
        fat_pool = ctx.enter_context(tc.tile_pool(name="fat", bufs=5))
        small = ctx.enter_context(tc.tile_pool(name="small", bufs=2))
        acc_pool = ctx.enter_context(tc.tile_pool(name="acc", bufs=1))
        psum_m = ctx.enter_context(tc.tile_pool(name="psum_m", bufs=2,
                                                space="PSUM"))

        # --- constants / whole-run inputs ---
        from concourse import library_config
        nc.gpsimd.load_library(library_config.ap_gather)

        w_sb = const.tile([128, K_PER, 2, 256], bf)      # [c_, k, kc, d]
        for kl in range(K_PER):
            for kc in range(2):
                nc.sync.dma_start(out=w_sb[:, kl, kc, :],
                                  in_=WT_d[kl, kc])
        b_sb = const.tile([128, K_PER, 2], f32)
        nc.sync.dma_start(out=b_sb, in_=b_d)
        gidx_sb = const.tile([128, S_PER, K_PER, U, NN], i16)
        nc.sync.dma_start(out=gidx_sb, in_=gidx_d)
        sel_sb = const.tile([128, FAT], f32)
        nc.sync.dma_start(out=sel_sb, in_=sel_d)
        sel3 = sel_sb.rearrange("p (n j) -> p n j", j=16)
        ones_sb = const.tile([128, 1], f32)
        nc.vector.memset(ones_sb, 1.0)

        partials = acc_pool.tile([128, 24], f32)
        nc.vector.memset(partials, 0.0)
        ssum_all = acc_pool.tile([128, S_PER * K_PER, U], f32)

        for s in range(S_PER):
            zt = {}
            ct = {}
            for dh in range(2):
                z_tile = zpool.tile([128, U, T], bf, tag=f"zT{dh}",
                                    name=f"z_s{s}_d{dh}")
                nc.sync.dma_start(out=z_tile, in_=zT_d[dh, :, s])
                zt[dh] = z_tile
                c_tile = zpool.tile([128, U, L], bf, tag=f"cT{dh}",
                                    name=f"c_s{s}_d{dh}")
                nc.sync.dma_start(out=c_tile, in_=cT_d[dh, :, s])
                ct[dh] = c_tile

            for kl in range(K_PER):
                # ---- stage 1: WcT[d-half, 1536 tokens] ----
                wc = {}
                for dh in range(2):
                    wt = wc_pool.tile([128, U * L], bf, tag=f"wc{dh}",
                                      name=f"wc_s{s}_k{kl}_d{dh}")
                    wc[dh] = wt
                    pmw = psum_m.tile([128, U * L], f32, tag="m",
                                      name=f"pwc_{s}_{kl}_{dh}")
                    for kc in range(2):
                        for nt in range(3):
                            rhs = (ct[kc].rearrange("p u t -> p (u t)")
                                   [:, nt * 512:(nt + 1) * 512])
                            nc.tensor.matmul(
                                pmw[:, nt * 512:(nt + 1) * 512],
                                lhsT=w_sb[:, kl, kc,
                                          dh * 128:(dh + 1) * 128],
                                rhs=rhs,
                                start=(kc == 0), stop=(kc == 1),
                            )
                    # bias-add + psum->sbuf (ACT, Identity w/ bias)
                    nc.scalar.activation(
                        wt, pmw,
                        mybir.ActivationFunctionType.Identity,
                        bias=b_sb[:, kl, dh:dh + 1],
                    )

                f_all = small.tile([128, U, NN], f32, tag="f_all",
                                   name=f"f_{s}_{kl}")
                # ---- stage 2 + extract per u (compact lags one u to
                # keep DVE's in-order queue from head-of-line blocking) ----
                pending = None
                for u in range(U):
                    pm = psum_m.tile([128, U * L], f32, tag="m",
                                     name=f"pm_{s}_{kl}_{u}")
                    for dh in range(2):
                        for bt in range(3):
                            rhs = zt[dh][:, bt * 4:(bt + 1) * 4,
                                         kl + 1:kl + 1 + L]
                            nc.tensor.matmul(
                                pm[:, bt * 512:(bt + 1) * 512],
                                lhsT=wc[dh][:, u * L:(u + 1) * L],
                                rhs=rhs,
                                start=(dh == 0), stop=(dh == 1),
                            )
                    m_sb = m_pool.tile([128, U * L], f32, tag="m_sb",
                                       name=f"m_{s}_{kl}_{u}")
                    nc.scalar.copy(m_sb[:, :ACT_COLS],
                                   pm[:, :ACT_COLS])
                    nc.vector.tensor_copy(m_sb[:, ACT_COLS:],
                                          pm[:, ACT_COLS:])
                    f_fat = fat_pool.tile([128, FAT], f32, tag="fat",
                                          name=f"ff_{s}_{kl}_{u}")
                    nc.gpsimd.ap_gather(
                        f_fat, m_sb, gidx_sb[:, s, kl, u, :],
                        channels=128, num_elems=U * L, d=1, num_idxs=FAT,
                    )
                    if pending is not None:
                        pu, pf = pending
                        fm = fat_pool.tile([128, NN, 16], f32, tag="fm",
                                           name=f"fm_{s}_{kl}_{pu}")
                        nc.vector.tensor_mul(
                            fm, pf.rearrange("p (n j) -> p n j", j=16), sel3)
                        nc.vector.reduce_sum(f_all[:, pu, :], fm,
                                             axis=mybir.AxisListType.X)
                    pending = (u, f_fat)
                pu, pf = pending
                fm = fat_pool.tile([128, NN, 16], f32, tag="fm",
                                   name=f"fm_{s}_{kl}_{pu}")
                nc.vector.tensor_mul(
                    fm, pf.rearrange("p (n j) -> p n j", j=16), sel3)
                nc.vector.reduce_sum(f_all[:, pu, :], fm,
                                     axis=mybir.AxisListType.X)

                # ---- softmax-CE over the NN classes, batched over u ----
                kcol = kl  # local k; host maps to global k
                fmax = small.tile([128, U], f32, tag="fmax",
                                  name=f"fmax_{s}_{kl}")
                nc.vector.reduce_max(fmax, f_all,
                                     axis=mybir.AxisListType.X)
                ts_t = small.tile([128, U, NN], f32, tag="ts",
                                  name=f"ts_{s}_{kl}")
                fmax_b = bass.AP(tensor=fmax.tensor, offset=fmax.offset,
                                 ap=[*fmax.ap, [0, NN]])
                nc.vector.tensor_sub(ts_t, f_all, fmax_b)
                e_t = small.tile([128, U, NN], f32, tag="e",
                                 name=f"e_{s}_{kl}")
                nc.scalar.activation(e_t, ts_t,
                                     mybir.ActivationFunctionType.Exp)
                unit = s * K_PER + kl
                nc.vector.reduce_sum(ssum_all[:, unit, :], e_t,
                                     axis=mybir.AxisListType.X)
                lp_s = small.tile([128, 1], f32, tag="lp_s",
                                  name=f"lps_{s}_{kl}")
                nc.vector.reduce_sum(lp_s, ts_t[:, :, 0],
                                     axis=mybir.AxisListType.X)
                nc.vector.tensor_add(partials[:, kcol:kcol + 1],
                                     partials[:, kcol:kcol + 1], lp_s)
                # acc: f0 >= max(negatives)
                nmax = small.tile([128, U], f32, tag="nmax",
                                  name=f"nm_{s}_{kl}")
                nc.vector.reduce_max(nmax, f_all[:, :, 1:],
                                     axis=mybir.AxisListType.X)
                ind = small.tile([128, U], f32, tag="ind",
                                 name=f"ind_{s}_{kl}")
                nc.vector.tensor_tensor(ind, f_all[:, :, 0], nmax,
                                        op=mybir.AluOpType.is_ge)
                ind_s = small.tile([128, 1], f32, tag="ind_s",
                                   name=f"inds_{s}_{kl}")
                nc.vector.reduce_sum(ind_s, ind,
                                     axis=mybir.AxisListType.X)
                nc.vector.tensor_add(partials[:, 12 + kcol:13 + kcol],
                                     partials[:, 12 + kcol:13 + kcol],
                                     ind_s)

        # ---- deferred Ln over all units' softmax sums ----
        lse_all = acc_pool.tile([128, S_PER * K_PER, U], f32)
        nc.scalar.activation(
            lse_all.rearrange("p a u -> p (a u)"),
            ssum_all.rearrange("p a u -> p (a u)"),
            mybir.ActivationFunctionType.Ln)
        for s in range(S_PER):
            for kl in range(K_PER):
                unit = s * K_PER + kl
                ls_s = small.tile([128, 1], f32, tag="ls_s",
                                  name=f"lss_{s}_{kl}")
                nc.vector.reduce_sum(ls_s, lse_all[:, unit, :],
                                     axis=mybir.AxisListType.X)
                nc.vector.tensor_sub(partials[:, kl:kl + 1],
                                     partials[:, kl:kl + 1], ls_s)

        # ---- final partition reduce: ones^T @ partials -> [1, 24] ----
        po = psum_m.tile([1, 24], f32, tag="m")
        nc.tensor.matmul(po, lhsT=ones_sb, rhs=partials,
                         start=True, stop=True)
        out_sb = small.tile([1, 24], f32, tag="out_sb")
        nc.vector.tensor_copy(out_sb, po)
        nc.sync.dma_start(out=out_d, in_=out_sb)

    nc.compile()
    return nc


def _prep_inputs(z, c, W, b, idx, num_negatives):
    """Build the 8 per-core input dicts (all numpy, host-side only)."""
    NN = num_negatives + 1
    FAT = NN * 16
    scale = np.float32(1.0 / np.sqrt(np.float32(D)))
    Ws = (W * scale).astype(np.float32)
    bs = (b * scale).astype(np.float32)

    sel16 = np.zeros((128, FAT), np.float32)
    pr = np.arange(128)
    for n in range(NN):
        sel16[pr, n * 16 + (pr % 16)] = 1.0

    in_maps = []
    for core in range(N_CORES):
        sg, kg = core // K_GRP, core % K_GRP
        sp = slice(sg * S_PER * U, (sg + 1) * S_PER * U)
        ks = slice(kg * K_PER, (kg + 1) * K_PER)
        # Pre-shift the time axis by the core's k-group offset so the
        # device's local-k window [kl+1, kl+1+L) lands on the global
        # window [kg*K_PER + kl + 1, ...).
        zsh = np.zeros((S_PER * U, T, D), z.dtype)
        zsh[:, :T - kg * K_PER] = z[sp, kg * K_PER:]
        zT = (zsh.reshape(S_PER, U, T, D).transpose(3, 0, 1, 2)
              .reshape(2, 128, S_PER, U, T))
        cT = (c[sp, :L].reshape(S_PER, U, L, C).transpose(3, 0, 1, 2)
              .reshape(2, 128, S_PER, U, L))
        WT = Ws[ks].transpose(0, 2, 1).reshape(K_PER, 2, 128, 256)
        bT = bs[ks].reshape(K_PER, 2, 128).transpose(2, 0, 1)
        gidx = idx[sg * S_PER:(sg + 1) * S_PER, ks].transpose(3, 0, 1, 2, 4)
        import ml_dtypes
        in_maps.append({
            "zT": np.ascontiguousarray(zT).astype(ml_dtypes.bfloat16),
            "cT": np.ascontiguousarray(cT).astype(ml_dtypes.bfloat16),
            "WT": np.ascontiguousarray(WT).astype(ml_dtypes.bfloat16),
            "bT": np.ascontiguousarray(bT),
            "gidx": np.ascontiguousarray(gidx),
            "sel16": sel16,
        })
    return in_maps


def _finalize(core_outs):
    # out_part columns use LOCAL k (0..5); map to global k via the core's
    # k-group: global_k = kg*K_PER + kl, kg = core % K_GRP.
    loss_k = np.zeros(K, np.float64)
    acc_k = np.zeros(K, np.float64)
    for core, o in enumerate(core_outs):
        kg = core % K_GRP
        v = np.asarray(o["out_part"], np.float64).reshape(24)
        loss_k[kg * K_PER:(kg + 1) * K_PER] += v[:K_PER]
        acc_k[kg * K_PER:(kg + 1) * K_PER] += v[12:12 + K_PER]
    loss = -loss_k.sum() / (K * S * U * L)
    accs = acc_k / (S * U * L)
    return np.float32(loss), accs.astype(np.float32)


def kernel(z, c, W, b, num_negatives):
    num_negatives = int(num_negatives)
    z = np.asarray(z, np.float32)
    c = np.asarray(c, np.float32)
    W = np.asarray(W, np.float32)
    b = np.asarray(b, np.float32)

    key = num_negatives
    if key not in _CACHE:
        idx = _build_indices(num_negatives)
        nc = _build_program(num_negatives)
        _CACHE[key] = (idx, nc)
    idx, nc = _CACHE[key]

    in_maps = _prep_inputs(z, c, W, b, idx, num_negatives)

    from concourse.bass_utils import run_bass_kernel_spmd
    res = run_bass_kernel_spmd(nc, in_maps, core_ids=list(range(N_CORES)))
    return _finalize(res.results)


if __name__ == "__main__":
    import reference
    inputs = reference.setup_inputs()
    el, ea = reference.reference(**inputs)
    al, aa = kernel(**inputs)
    print("loss:", al, "exp:", float(el),
          "relerr:", abs(al - float(el)) / abs(float(el)))
    print("accs maxrelerr:",
          np.abs(aa - np.asarray(ea)).max() / np.abs(np.asarray(ea)).max())


# revision 14
# speedup vs baseline: 1.0076x; 1.0054x over previous
"""CPC predictor loss kernel for 8x Trainium2 NeuronCores (Bass/Tile).

Strategy
--------
B=144 = 12 speakers x 12 utts; K=12 prediction steps; L=128; D=C=256.
Negatives are sampled within-speaker, so all scores for one speaker come
from the within-speaker Gram-style matrix

    M_su[l, b*128+j] = Wc[s,u,l,:] . z4[s,b,j,:]   (Wc = c @ W_k^T + b_k)

Work is sharded over 8 cores as 4 speaker-groups (3 speakers) x 2
k-groups (6 steps) = 18 (s,k) units per core, zero collectives.

Per (s,k) unit on-device:
  stage 1 (PE):   WcT[d, (u,l)] = W_k^T-tiles @ cT   (fp32r matmuls)
  stage 2 (PE):   M_su = WcT_u^T @ z-window-T          (fp32r matmuls)
  extract (GPSIMD ap_gather): per-16-partition-group index lists pull the
      11 class scores per row into a "fat" [128, 11*16] tile
  compact (DVE):  mask-multiply by sel16 + reduce -> f[128, 11]
  CE (DVE+ACT):   log-softmax over the 11 classes, argmax==0 indicator,
      accumulated into per-partition partials [128, 24]
  final:          ones^T @ partials matmul -> [1,24] -> DRAM

Host: pre-transposes all inputs into SBUF-ready layouts, precomputes the
RNG-derived gather indices (jax threefry on CPU - identical to the
reference's), and sums the 8 per-core partial vectors.
"""

import numpy as np

S, U, K, L, D, C, T = 12, 12, 12, 128, 256, 256, 140
N_CORES = 8
S_GRP, K_GRP = 4, 2          # core = sg * K_GRP + kg
S_PER, K_PER = 3, 6          # speakers / k-steps per core
ACT_COLS = 896               # M psum->sbuf copy: ACT [0:ACT_COLS), DVE rest

_CACHE: dict = {}


def _build_indices(num_negatives: int):
    """Reproduce the reference's negative-sampling indices (jax threefry)."""
    import jax
    import jax.numpy as jnp

    cpu = jax.devices("cpu")[0]
    with jax.default_device(cpu):
        base = jax.random.key(42)
        bidx_all = np.zeros((K, U, num_negatives), np.int64)
        sidx_all = np.zeros((K, S, U, num_negatives, L), np.int64)
        for k in range(K):
            kb, ks = jax.random.split(jax.random.fold_in(base, k))
            bidx = jax.random.randint(kb, (U, num_negatives), 0, U)
            sidx = jax.random.randint(ks, (S, U, num_negatives, L), 0, L)
            sidx = (sidx + jnp.arange(L)) % L
            bidx_all[k] = np.asarray(bidx)
            sidx_all[k] = np.asarray(sidx)

    # gather columns: col(s,k,u,l,0) = u*L + l (positive);
    # col(s,k,u,l,1+n) = bidx[k,u,n]*L + sidx[k,s,u,n,l]
    NN = num_negatives + 1
    idx = np.zeros((S, K, U, L, NN), np.int16)
    lr = np.arange(L)
    for s in range(S):
        for k in range(K):
            for u in range(U):
                idx[s, k, u, :, 0] = u * L + lr
                idx[s, k, u, :, 1:] = (
                    bidx_all[k, u][None, :] * L + sidx_all[k, s, u].T
                )
    return idx


def _build_program(num_negatives: int):
    import concourse.bass as bass
    import concourse.bacc as bacc
    import concourse.tile as tile
    from concourse import mybir
    from contextlib import ExitStack

    NN = num_negatives + 1
    FAT = NN * 16
    f32 = mybir.dt.float32
    f32r = mybir.dt.float32r
    bf = mybir.dt.bfloat16
    i16 = mybir.dt.int16

    nc = bacc.Bacc("TRN2", target_bir_lowering=False, debug=False,
                   num_devices=N_CORES)

    zT_d = nc.dram_tensor("zT", (2, 128, S_PER, U, T), bf,
                          kind="ExternalInput").ap()
    cT_d = nc.dram_tensor("cT", (2, 128, S_PER, U, L), bf,
                          kind="ExternalInput").ap()
    WT_d = nc.dram_tensor("WT", (K_PER, 2, 128, 256), bf,
                          kind="ExternalInput").ap()
    b_d = nc.dram_tensor("bT", (128, K_PER, 2), f32,
                         kind="ExternalInput").ap()
    gidx_d = nc.dram_tensor("gidx", (128, S_PER, K_PER, U, NN), i16,
                            kind="ExternalInput").ap()
    sel_d = nc.dram_tensor("sel16", (128, FAT), f32,
                           kind="ExternalInput").ap()
    out_d = nc.dram_tensor("out_part", (1, 24), f32,
                           kind="ExternalOutput").ap()

    with ExitStack() as ctx:
        tc = ctx.enter_context(tile.TileContext(nc))
        const = ctx.enter_context(tc.tile_pool(name="const", bufs=1))
        zpool = ctx.enter_context(tc.tile_pool(name="zpool", bufs=2))
        wc_pool = ctx.enter_context(tc.tile_pool(name="wc", bufs=2))
        m_pool = ctx.enter_context(tc.tile_pool(name="m", bufs=4))
        fat_pool = ctx.enter_context(tc.tile_pool(name="fat", bufs=5))
        small = ctx.enter_context(tc.tile_pool(name="small", bufs=2))
        acc_pool = ctx.enter_context(tc.tile_pool(name="acc", bufs=1))
        psum_m = ctx.enter_context(tc.tile_pool(name="psum_m", bufs=6,
                                                space="PSUM"))

        # --- constants / whole-run inputs ---
        from concourse import library_config
        nc.gpsimd.load_library(library_config.ap_gather)

        w_sb = const.tile([128, K_PER, 2, 256], bf)      # [c_, k, kc, d]
        for kl in range(K_PER):
            for kc in range(2):
                nc.sync.dma_start(out=w_sb[:, kl, kc, :],
                                  in_=WT_d[kl, kc])
        b_sb = const.tile([128, K_PER, 2], f32)
        nc.sync.dma_start(out=b_sb, in_=b_d)
        gidx_sb = const.tile([128, S_PER, K_PER, U, NN], i16)
        nc.sync.dma_start(out=gidx_sb, in_=gidx_d)
        sel_sb = const.tile([128, FAT], f32)
        nc.sync.dma_start(out=sel_sb, in_=sel_d)
        sel3 = sel_sb.rearrange("p (n j) -> p n j", j=16)
        ones_sb = const.tile([128, 1], f32)
        nc.vector.memset(ones_sb, 1.0)

        partials = acc_pool.tile([128, 24], f32)
        nc.vector.memset(partials, 0.0)
        ssum_all = acc_pool.tile([128, S_PER * K_PER, U], f32)

        for s in range(S_PER):
            zt = {}
            ct = {}
            for dh in range(2):
                z_tile = zpool.tile([128, U, T], bf, tag=f"zT{dh}",
                                    name=f"z_s{s}_d{dh}")
                nc.sync.dma_start(out=z_tile, in_=zT_d[dh, :, s])
                zt[dh] = z_tile
                c_tile = zpool.tile([128, U, L], bf, tag=f"cT{dh}",
                                    name=f"c_s{s}_d{dh}")
                nc.sync.dma_start(out=c_tile, in_=cT_d[dh, :, s])
                ct[dh] = c_tile

            for kl in range(K_PER):
                # ---- stage 1: WcT[d-half, 1536 tokens] ----
                wc = {}
                for dh in range(2):
                    wt = wc_pool.tile([128, U * L], bf, tag=f"wc{dh}",
                                      name=f"wc_s{s}_k{kl}_d{dh}")
                    wc[dh] = wt
                    pws = [psum_m.tile([128, 512], f32, tag="m",
                                       name=f"pwc_{s}_{kl}_{dh}_{nt}")
                           for nt in range(3)]
                    for kc in range(2):
                        for nt in range(3):
                            rhs = (ct[kc].rearrange("p u t -> p (u t)")
                                   [:, nt * 512:(nt + 1) * 512])
                            nc.tensor.matmul(
                                pws[nt],
                                lhsT=w_sb[:, kl, kc,
                                          dh * 128:(dh + 1) * 128],
                                rhs=rhs,
                                start=(kc == 0), stop=(kc == 1),
                            )
                    for nt in range(3):
                        # bias-add + psum->sbuf (ACT, Identity w/ bias)
                        nc.scalar.activation(
                            wt[:, nt * 512:(nt + 1) * 512], pws[nt],
                            mybir.ActivationFunctionType.Identity,
                            bias=b_sb[:, kl, dh:dh + 1],
                        )

                f_all = small.tile([128, U, NN], f32, tag="f_all",
                                   name=f"f_{s}_{kl}")
                # ---- stage 2 + extract per u (compact lags one u to
                # keep DVE's in-order queue from head-of-line blocking) ----
                pending = None
                for u in range(U):
                    m_sb = m_pool.tile([128, U * L], f32, tag="m_sb",
                                       name=f"m_{s}_{kl}_{u}")
                    for bt in range(3):
                        pb = psum_m.tile([128, 512], f32, tag="m",
                                         name=f"pm_{s}_{kl}_{u}_{bt}")
                        for dh in range(2):
                            rhs = zt[dh][:, bt * 4:(bt + 1) * 4,
                                         kl + 1:kl + 1 + L]
                            nc.tensor.matmul(
                                pb,
                                lhsT=wc[dh][:, u * L:(u + 1) * L],
                                rhs=rhs,
                                start=(dh == 0), stop=(dh == 1),
                            )
                        dst = m_sb[:, bt * 512:(bt + 1) * 512]
                        if bt == 2:
                            nc.vector.tensor_copy(dst, pb)
                        else:
                            nc.scalar.copy(dst, pb)
                    f_fat = fat_pool.tile([128, FAT], f32, tag="fat",
                                          name=f"ff_{s}_{kl}_{u}")
                    nc.gpsimd.ap_gather(
                        f_fat, m_sb, gidx_sb[:, s, kl, u, :],
                        channels=128, num_elems=U * L, d=1, num_idxs=FAT,
                    )
                    if pending is not None:
                        pu, pf = pending
                        fm = fat_pool.tile([128, NN, 16], f32, tag="fm",
                                           name=f"fm_{s}_{kl}_{pu}")
                        nc.vector.tensor_mul(
                            fm, pf.rearrange("p (n j) -> p n j", j=16), sel3)
                        nc.vector.reduce_sum(f_all[:, pu, :], fm,
                                             axis=mybir.AxisListType.X)
                    pending = (u, f_fat)
                pu, pf = pending
                fm = fat_pool.tile([128, NN, 16], f32, tag="fm",
                                   name=f"fm_{s}_{kl}_{pu}")
                nc.vector.tensor_mul(
                    fm, pf.rearrange("p (n j) -> p n j", j=16), sel3)
                nc.vector.reduce_sum(f_all[:, pu, :], fm,
                                     axis=mybir.AxisListType.X)

                # ---- softmax-CE over the NN classes, batched over u ----
                kcol = kl  # local k; host maps to global k
                fmax = small.tile([128, U], f32, tag="fmax",
                                  name=f"fmax_{s}_{kl}")
                nc.vector.reduce_max(fmax, f_all,
                                     axis=mybir.AxisListType.X)
                ts_t = small.tile([128, U, NN], f32, tag="ts",
                                  name=f"ts_{s}_{kl}")
                fmax_b = bass.AP(tensor=fmax.tensor, offset=fmax.offset,
                                 ap=[*fmax.ap, [0, NN]])
                nc.vector.tensor_sub(ts_t, f_all, fmax_b)
                e_t = small.tile([128, U, NN], f32, tag="e",
                                 name=f"e_{s}_{kl}")
                nc.scalar.activation(e_t, ts_t,
                                     mybir.ActivationFunctionType.Exp)
                unit = s * K_PER + kl
                nc.vector.reduce_sum(ssum_all[:, unit, :], e_t,
                                     axis=mybir.AxisListType.X)
                lp_s = small.tile([128, 1], f32, tag="lp_s",
                                  name=f"lps_{s}_{kl}")
                nc.vector.reduce_sum(lp_s, ts_t[:, :, 0],
                                     axis=mybir.AxisListType.X)
                nc.vector.tensor_add(partials[:, kcol:kcol + 1],
                                     partials[:, kcol:kcol + 1], lp_s)
                # acc: f0 >= max(negatives)
                nmax = small.tile([128, U], f32, tag="nmax",
                                  name=f"nm_{s}_{kl}")
                nc.vector.reduce_max(nmax, f_all[:, :, 1:],
                                     axis=mybir.AxisListType.X)
                ind = small.tile([128, U], f32, tag="ind",
                                 name=f"ind_{s}_{kl}")
                nc.vector.tensor_tensor(ind, f_all[:, :, 0], nmax,
                                        op=mybir.AluOpType.is_ge)
                ind_s = small.tile([128, 1], f32, tag="ind_s",
                                   name=f"inds_{s}_{kl}")
                nc.vector.reduce_sum(ind_s, ind,
                                     axis=mybir.AxisListType.X)
                nc.vector.tensor_add(partials[:, 12 + kcol:13 + kcol],
                                     partials[:, 12 + kcol:13 + kcol],
                                     ind_s)

        # ---- deferred Ln over all units' softmax sums ----
        lse_all = acc_pool.tile([128, S_PER * K_PER, U], f32)
        nc.scalar.activation(
            lse_all.rearrange("p a u -> p (a u)"),
            ssum_all.rearrange("p a u -> p (a u)"),
            mybir.ActivationFunctionType.Ln)
        for s in range(S_PER):
            for kl in range(K_PER):
                unit = s * K_PER + kl
                ls_s = small.tile([128, 1], f32, tag="ls_s",
                                  name=f"lss_{s}_{kl}")
                nc.vector.reduce_sum(ls_s, lse_all[:, unit, :],
                                     axis=mybir.AxisListType.X)
                nc.vector.tensor_sub(partials[:, kl:kl + 1],
                                     partials[:, kl:kl + 1], ls_s)

        # ---- final partition reduce: ones^T @ partials -> [1, 24] ----
        po = psum_m.tile([1, 24], f32, tag="m")
        nc.tensor.matmul(po, lhsT=ones_sb, rhs=partials,
                         start=True, stop=True)
        out_sb = small.tile([1, 24], f32, tag="out_sb")
        nc.vector.tensor_copy(out_sb, po)
        nc.sync.dma_start(out=out_d, in_=out_sb)

    nc.compile()
    return nc


def _prep_inputs(z, c, W, b, idx, num_negatives):
    """Build the 8 per-core input dicts (all numpy, host-side only)."""
    NN = num_negatives + 1
    FAT = NN * 16
    scale = np.float32(1.0 / np.sqrt(np.float32(D)))
    Ws = (W * scale).astype(np.float32)
    bs = (b * scale).astype(np.float32)

    sel16 = np.zeros((128, FAT), np.float32)
    pr = np.arange(128)
    for n in range(NN):
        sel16[pr, n * 16 + (pr % 16)] = 1.0

    in_maps = []
    for core in range(N_CORES):
        sg, kg = core // K_GRP, core % K_GRP
        sp = slice(sg * S_PER * U, (sg + 1) * S_PER * U)
        ks = slice(kg * K_PER, (kg + 1) * K_PER)
        # Pre-shift the time axis by the core's k-group offset so the
        # device's local-k window [kl+1, kl+1+L) lands on the global
        # window [kg*K_PER + kl + 1, ...).
        zsh = np.zeros((S_PER * U, T, D), z.dtype)
        zsh[:, :T - kg * K_PER] = z[sp, kg * K_PER:]
        zT = (zsh.reshape(S_PER, U, T, D).transpose(3, 0, 1, 2)
              .reshape(2, 128, S_PER, U, T))
        cT = (c[sp, :L].reshape(S_PER, U, L, C).transpose(3, 0, 1, 2)
              .reshape(2, 128, S_PER, U, L))
        WT = Ws[ks].transpose(0, 2, 1).reshape(K_PER, 2, 128, 256)
        bT = bs[ks].reshape(K_PER, 2, 128).transpose(2, 0, 1)
        gidx = idx[sg * S_PER:(sg + 1) * S_PER, ks].transpose(3, 0, 1, 2, 4)
        import ml_dtypes
        in_maps.append({
            "zT": np.ascontiguousarray(zT).astype(ml_dtypes.bfloat16),
            "cT": np.ascontiguousarray(cT).astype(ml_dtypes.bfloat16),
            "WT": np.ascontiguousarray(WT).astype(ml_dtypes.bfloat16),
            "bT": np.ascontiguousarray(bT),
            "gidx": np.ascontiguousarray(gidx),
            "sel16": sel16,
        })
    return in_maps


def _finalize(core_outs):
    # out_part columns use LOCAL k (0..5); map to global k via the core's
    # k-group: global_k = kg*K_PER + kl, kg = core % K_GRP.
    loss_k = np.zeros(K, np.float64)
    acc_k = np.zeros(K, np.float64)
    for core, o in enumerate(core_outs):
        kg = core % K_GRP
        v = np.asarray(o["out_part"], np.float64).reshape(24)
        loss_k[kg * K_PER:(kg + 1) * K_PER] += v[:K_PER]
        acc_k[kg * K_PER:(kg + 1) * K_PER] += v[12:12 + K_PER]
    loss = -loss_k.sum() / (K * S * U * L)
    accs = acc_k / (S * U * L)
    return np.float32(loss), accs.astype(np.float32)


def kernel(z, c, W, b, num_negatives):
    num_negatives = int(num_negatives)
    z = np.asarray(z, np.float32)
    c = np.asarray(c, np.float32)
    W = np.asarray(W, np.float32)
    b = np.asarray(b, np.float32)

    key = num_negatives
    if key not in _CACHE:
        idx = _build_indices(num_negatives)
        nc = _build_program(num_negatives)
        _CACHE[key] = (idx, nc)
    idx, nc = _CACHE[key]

    in_maps = _prep_inputs(z, c, W, b, idx, num_negatives)

    from concourse.bass_utils import run_bass_kernel_spmd
    res = run_bass_kernel_spmd(nc, in_maps, core_ids=list(range(N_CORES)))
    return _finalize(res.results)


if __name__ == "__main__":
    import reference
    inputs = reference.setup_inputs()
    el, ea = reference.reference(**inputs)
    al, aa = kernel(**inputs)
    print("loss:", al, "exp:", float(el),
          "relerr:", abs(al - float(el)) / abs(float(el)))
    print("accs maxrelerr:",
          np.abs(aa - np.asarray(ea)).max() / np.abs(np.asarray(ea)).max())
